# revision 19
# baseline (speedup 1.0000x reference)
"""Distributed Trainium2 Bass kernel for the reference attention block.

Shapes: x[2, 2048, 1024], 16 heads x 64 dim, RoPE, additive mask, softmax,
out_proj.  Sharding over 8 NeuronCores: core c = (batch b = c // 4,
head-group hg = c % 4 of 4 heads).  Per core: QKV projection for its 4 heads
(column-parallel), RoPE, two-pass flash-style attention, partial out_proj
(row-parallel), then ReduceScatter(add) over the 4 cores of the same batch.
Host concatenates the per-core [512, 1024] output shards.

Numerical structure:
  * pass A computes S = (q/8) K^T tile-rows [i, j] only to extract row maxes m.
  * pass B computes S^T [j, i] with an augmented contraction: Q_aug has a 65th
    row holding -m, K_aug a 65th row of ones, so the matmul directly yields
    S^T - m.  exp() on ACT, then the context matmul with V_aug (65th column of
    ones) accumulates both the context numerator and the softmax denominator.
  * mask handling is block-wise: the host classifies each 128x128 mask tile as
    SKIP (<= -1e8 everywhere), FREE (all zeros) or MASKED, merged over both
    batches so all 8 cores run one SPMD graph.  SKIP blocks are never computed
    (a causal mask skips ~half the attention work); only MASKED blocks move
    mask bytes.
"""

import sys

for _p in ("/opt/trn_rl_repo",):
    if _p not in sys.path:
        sys.path.insert(0, _p)

import numpy as np
import ml_dtypes

import concourse.bass as bass
import concourse.mybir as mybir
import concourse.tile as tile
from concourse import bacc
from concourse.bass_utils import run_bass_kernel_spmd
from concourse.masks import make_identity

B, T, C = 2, 2048, 1024
H, D = 16, 64
NCORES = 8
GROUPS = [[0, 1, 2, 3], [4, 5, 6, 7]]
HPC = 4                  # heads per core
FPC = HPC * D            # 256 projected features per core (per q/k/v)
NT = T // 128            # 16 row tiles
NIB = T // 512           # 4 query blocks in pass B
BF16 = mybir.dt.bfloat16
F32 = mybir.dt.float32
NPBF16 = ml_dtypes.bfloat16

SKIP, FREE, MASKED = 0, 1, 2


def _analyze_mask(attn_mask):
    """Merged 128x128 block flags across both batches (one SPMD graph)."""
    tiles = attn_mask.reshape(B, NT, 128, NT, 128)
    skip = (tiles <= -1e8).all(axis=(2, 4))     # [B, NT, NT]
    free = (tiles == 0).all(axis=(2, 4))
    flags = np.full((NT, NT), MASKED, dtype=np.int8)
    flags[free.all(axis=0)] = FREE
    flags[skip.all(axis=0)] = SKIP
    for it in range(NT):                        # fully-masked query rows:
        if (flags[it] == SKIP).all():           # compute them masked so the
            flags[it] = MASKED                  # softmax matches the reference
    return flags


def _plan(flags):
    """Static loop structure shared by every core.

    passA[it] = runs (j0_tile, n_tiles, [masked_offsets]); each run is a
    contiguous stretch of <=4 non-SKIP key tiles.
    passB[ib] = list of (jt, needs_mask) for the 512-wide query block ib.
    """
    passA = []
    for it in range(NT):
        runs = []
        jt = 0
        while jt < NT:
            if flags[it, jt] == SKIP:
                jt += 1
                continue
            j0 = jt
            while jt < NT and jt - j0 < 4 and flags[it, jt] != SKIP:
                jt += 1
            masked = [k - j0 for k in range(j0, jt) if flags[it, k] == MASKED]
            runs.append((j0, jt - j0, masked))
        passA.append(runs)

    passB = []
    for ib in range(NIB):
        sub = flags[ib * 4:(ib + 1) * 4]        # [4, NT]
        blocks = []
        for jt in range(NT):
            col = sub[:, jt]
            if (col == SKIP).all():
                continue
            blocks.append((jt, not (col == FREE).all()))
        passB.append(blocks)
    return passA, passB


def _build_graph(flags, mfree=False, debug=False):
    passA, passB = _plan(flags)
    nA = sum(len(m) for runs in passA for (_, _, m) in runs)
    nB = sum(1 for blocks in passB for (_, msk) in blocks if msk)

    nc = bacc.Bacc(num_devices=NCORES)

    # ---- parameters (per-core shards, prepared on host) ----
    p_xT = nc.declare_dram_parameter("xT", [C, T], BF16, isOutput=False)
    p_wqkT = nc.declare_dram_parameter("wqkT", [C, 2 * FPC], BF16, isOutput=False)
    p_wvT = nc.declare_dram_parameter("wvT", [C, FPC], BF16, isOutput=False)
    p_qkb = nc.declare_dram_parameter("qkb", [1, 2 * FPC], BF16, isOutput=False)
    p_vb = nc.declare_dram_parameter("vb", [1, FPC], BF16, isOutput=False)
    p_ct = nc.declare_dram_parameter("ct", [128, T], BF16, isOutput=False)
    p_st = nc.declare_dram_parameter("st", [128, T], BF16, isOutput=False)
    p_w0 = nc.declare_dram_parameter("wout0", [128, C], BF16, isOutput=False)
    p_w1 = nc.declare_dram_parameter("wout1", [128, C], BF16, isOutput=False)
    p_ob = nc.declare_dram_parameter("obias", [1, C], BF16, isOutput=False)
    p_mA = nc.declare_dram_parameter("maskA", [max(nA, 1), 128, 128], F32,
                                     isOutput=False)
    p_mB = nc.declare_dram_parameter("maskB", [max(nB, 1), 128, 512], F32,
                                     isOutput=False)
    p_out = nc.declare_dram_parameter("out", [T // 4, C], BF16, isOutput=True)
    if debug:
        p_dqa = nc.declare_dram_parameter("dqa", [HPC, 65, T], BF16, isOutput=True)
        p_dka = nc.declare_dram_parameter("dka", [HPC, 65, T], BF16, isOutput=True)
        p_dmall = nc.declare_dram_parameter("dmall", [128, HPC * NT], F32,
                                            isOutput=True)
        p_dot = nc.declare_dram_parameter("dot", [2, 128, T], BF16, isOutput=True)
        p_dva = nc.declare_dram_parameter("dva", [NT, 128, HPC * 65], BF16,
                                          isOutput=True)


    with tile.TileContext(nc) as tc, \
            tc.tile_pool(name="static", bufs=1) as st_pool, \
            tc.tile_pool(name="sdram", bufs=1, space="DRAM") as dr_pool:
        def _t(shape, dtype, name, **k):
            return st_pool.tile(shape, dtype, name=name, tag=name, **k)

        # ---- static SBUF tensors ----
        xT = [_t([128, T], BF16, name=f"xT{i}") for i in range(8)]
        wqk = [_t([128, 2 * FPC], BF16, name=f"wqk{i}") for i in range(8)]
        wv = [_t([128, FPC], BF16, name=f"wv{i}") for i in range(8)]
        qkb = _t([1, 2 * FPC], BF16, name="qkb")
        vb = _t([1, FPC], BF16, name="vb")
        ct = _t([128, T], BF16, name="ct")
        st = _t([128, T], BF16, name="st")
        w0 = _t([128, C], BF16, name="w0")
        w1 = _t([128, C], BF16, name="w1")
        obias = _t([1, C], BF16, name="obias")
        # Q/K augmented: rows 0..63 = RoPE'd head dims, row 64 = -m (Q), 1s (K)
        qa = [_t([65, T], BF16, name=f"qa{h}") for h in range(HPC)]
        ka = [_t([65, T], BF16, name=f"ka{h}") for h in range(HPC)]
        # V augmented per key tile: [128, 4 heads x (64 dims + ones col)]
        va = [_t([128, HPC * 65], BF16, name=f"va{j}") for j in range(NT)]
        # context output, [dv, t] layout, two 128-row chunks
        ot = [_t([128, T], BF16, name=f"ot{i}") for i in range(2)]
        mall = _t([128, HPC * NT], F32, name="mall")   # running row maxes
        ident = _t([128, 128], F32, name="ident")
        ones65 = _t([65, 64], F32, name="ones65")
        ones_t = _t([1, 512], BF16, name="ones_t")

        make_identity(nc, ident[:, :])
        nc.gpsimd.memset(ones65[:, :], 1.0)
        nc.vector.memset(ones_t[:, :], 1.0)

        for i in range(8):
            nc.sync.dma_start(out=xT[i][:, :], in_=p_xT[i * 128:(i + 1) * 128, :])
            nc.sync.dma_start(out=wqk[i][:, :], in_=p_wqkT[i * 128:(i + 1) * 128, :])
            nc.sync.dma_start(out=wv[i][:, :], in_=p_wvT[i * 128:(i + 1) * 128, :])
        nc.sync.dma_start(out=qkb[:, :], in_=p_qkb[:, :])
        nc.sync.dma_start(out=vb[:, :], in_=p_vb[:, :])
        for sb, pp in ((ct, p_ct), (st, p_st),
                       (w0, p_w0), (w1, p_w1), (obias, p_ob)):
            nc.sync.dma_start(out=sb[:, :], in_=pp[:, :])

        with (
            tc.tile_pool(name="ps_big", bufs=3, space="PSUM") as ps_big,
            tc.tile_pool(name="ps_ot", bufs=2, space="PSUM") as ps_ot,
            tc.tile_pool(name="ps_rep", bufs=2, space="PSUM") as ps_rep,
            tc.tile_pool(name="sb_raw", bufs=2) as sb_raw,
            tc.tile_pool(name="sb_tmp", bufs=2) as sb_tmp,
            tc.tile_pool(name="sb_et", bufs=4) as sb_et,
            tc.tile_pool(name="sb_msk", bufs=3) as sb_msk,
            tc.tile_pool(name="sb_st", bufs=4) as sb_st,
        ):
            # ================= QKV projection + RoPE =================
            # q/k: psum[f, t] = wqk^T x (+bias); f = 2 heads per M-tile.
            for mt in range(4):            # 0,1: q heads 01/23; 2,3: k heads
                raw = sb_raw.tile([128, T], BF16, tag="raw")
                for tb in range(4):
                    ps = ps_big.tile([128, 512], F32, tag="big")
                    for kc in range(8):
                        nc.tensor.matmul(
                            ps[:, :], wqk[kc][:, mt * 128:(mt + 1) * 128],
                            xT[kc][:, tb * 512:(tb + 1) * 512],
                            start=(kc == 0), stop=False)
                    nc.tensor.matmul(
                        ps[:, :], qkb[:, mt * 128:(mt + 1) * 128],
                        ones_t[:, :], start=False, stop=True)
                    nc.scalar.copy(raw[:, tb * 512:(tb + 1) * 512], ps[:, :])
                tgt = qa if mt < 2 else ka
                rawrot = sb_raw.tile([128, T], BF16, tag="rawrot", bufs=1)
                for s in range(2):
                    r = s * 64
                    nc.sync.dma_start(out=rawrot[r:r + 32, :],
                                      in_=raw[r + 32:r + 64, :])
                    nc.sync.dma_start(out=rawrot[r + 32:r + 64, :],
                                      in_=raw[r:r + 32, :])
                tmpA = sb_tmp.tile([128, T], BF16, tag="tmpA", bufs=1)
                qk2 = sb_tmp.tile([128, T], BF16, tag="qk2", bufs=1)
                nc.vector.tensor_mul(tmpA[:, :], raw[:, :], ct[:, :])
                nc.vector.tensor_mul(qk2[:, :], rawrot[:, :], st[:, :])
                nc.vector.tensor_add(qk2[:, :], tmpA[:, :], qk2[:, :])
                for s in range(2):
                    h = (mt % 2) * 2 + s
                    r = s * 64
                    nc.sync.dma_start(out=tgt[h][0:64, :], in_=qk2[r:r + 64, :])
            for h in range(HPC):           # K ones row
                nc.gpsimd.memset(ka[h][64:65, :], 1.0)

            # v: psum[t, dv] = x^T wv (+bias), packed into va with ones cols.
            for tt in range(NT):
                ps = ps_big.tile([128, 512], F32, tag="big")
                for kc in range(8):
                    nc.tensor.matmul(
                        ps[:, 0:FPC], xT[kc][:, tt * 128:(tt + 1) * 128],
                        wv[kc][:, :], start=(kc == 0), stop=False)
                nc.tensor.matmul(ps[:, 0:FPC], ones_t[:1, 0:128], vb[:, :],
                                 start=False, stop=True)
                nc.vector.memset(va[tt][:, :], 1.0)
                nc.scalar.copy(
                    va[tt][:, :].rearrange("p (h e) -> p h e", e=65)[:, :, 0:64],
                    ps[:, 0:FPC].rearrange("p (h d) -> p h d", d=64))

            # ================= pass A (all heads): row maxes =================
            # (skipped when the host-computed Cauchy-Schwarz score bound
            #  shows exp() cannot overflow/underflow: qa row 64 stays 0)
            ia = 0
            for h in range(HPC if not mfree else 0):
                for it in range(NT):
                    col = h * NT + it
                    first = True
                    for (j0, njt, masked) in passA[it]:
                        ln = njt * 128
                        ps = ps_big.tile([128, 512], F32, tag="big")
                        nc.tensor.matmul(
                            ps[:, :ln], qa[h][0:64, it * 128:(it + 1) * 128],
                            ka[h][0:64, j0 * 128:j0 * 128 + ln],
                            start=True, stop=True)
                        for off in masked:
                            msk = sb_msk.tile([128, 128], F32, tag="mA")
                            nc.sync.dma_start(out=msk[:, :],
                                              in_=p_mA[ia % max(nA, 1)])
                            ia += 1
                            nc.vector.tensor_add(
                                ps[:, off * 128:(off + 1) * 128],
                                ps[:, off * 128:(off + 1) * 128], msk[:, :])
                        if first:
                            nc.vector.reduce_max(
                                mall[:, col:col + 1], ps[:, :ln],
                                axis=mybir.AxisListType.X)
                            first = False
                        else:
                            mtmp = sb_st.tile([128, 1], F32, tag="mtmp")
                            nc.vector.reduce_max(
                                mtmp[:, :], ps[:, :ln],
                                axis=mybir.AxisListType.X)
                            nc.vector.tensor_max(
                                mall[:, col:col + 1], mall[:, col:col + 1],
                                mtmp[:, :])
                # transpose this head's maxes to a row, negate into q row 64
                pmt = ps_rep.tile([64, 512], F32, tag="rep")
                nc.tensor.transpose(pmt[0:NT, 0:128],
                                    mall[:, h * NT:(h + 1) * NT], ident[:, :])
                msb = sb_st.tile([16, 128], BF16, tag="msb")
                nc.scalar.activation(msb[:, :], pmt[0:NT, 0:128],
                                     mybir.ActivationFunctionType.Copy,
                                     scale=-1.0)
                nc.sync.dma_start(out=qa[h][64:65, :], in_=msb[:, :])
            if mfree:
                for h in range(HPC):
                    nc.gpsimd.memset(qa[h][64:65, :], 0.0)

            # ======== pass B + out_proj + chunked ReduceScatter ========
            rs_in = [dr_pool.tile([512, C], BF16, name=f"rs_in{g}",
                                  tag=f"rs_in{g}") for g in range(NIB)]
            rs_out = [dr_pool.tile([128, C], BF16, name=f"rs_out{g}",
                                   tag=f"rs_out{g}") for g in range(NIB)]
            mb_idx = {}
            for _ib in range(NIB):
                for (_jt, _mf) in passB[_ib]:
                    if _mf:
                        mb_idx[(_ib, _jt)] = len(mb_idx)
            for ib in range(NIB):
                blocks = passB[ib]
                for h in range(HPC):
                    po = ps_ot.tile([65, 512], F32, tag="ot")
                    for bi, (jt, msk_flag) in enumerate(blocks):
                        ps = ps_big.tile([128, 512], F32, tag="big")
                        nc.tensor.matmul(
                            ps[:, :], ka[h][0:65, jt * 128:(jt + 1) * 128],
                            qa[h][0:65, ib * 512:(ib + 1) * 512],
                            start=True, stop=True)
                        if msk_flag:
                            mskb = sb_msk.tile([128, 512], F32, tag="mB")
                            nc.sync.dma_start(out=mskb[:, :],
                                              in_=p_mB[mb_idx[(ib, jt)]])
                            nc.vector.tensor_add(ps[:, :], ps[:, :],
                                                 mskb[:, :])
                        et = sb_et.tile([128, 512], BF16, tag="et")
                        nc.scalar.activation(et[:, :], ps[:, :],
                                             mybir.ActivationFunctionType.Exp)
                        nc.tensor.matmul(
                            po[:, :], va[jt][:, h * 65:(h + 1) * 65], et[:, :],
                            start=(bi == 0), stop=(bi == len(blocks) - 1))
                    linv65 = sb_st.tile([65, 512], F32, tag="linv65", bufs=2)
                    nc.vector.reciprocal(linv65[64:65, :], po[64:65, :])
                    prep_ps = ps_rep.tile([64, 512], F32, tag="rep")
                    nc.tensor.matmul(prep_ps[:, :], ones65[64:65, 0:64],
                                     linv65[64:65, :], start=True, stop=True)
                    prep = sb_st.tile([64, 512], F32, tag="prep", bufs=2)
                    nc.scalar.copy(prep[:, :], prep_ps[:, :])
                    if h % 2 == 0:
                        nc.vector.tensor_mul(
                            ot[h // 2][0:64, ib * 512:(ib + 1) * 512],
                            po[0:64, :], prep[:, :])
                    else:
                        otmp = sb_st.tile([64, 512], BF16, tag="otmp")
                        nc.vector.tensor_mul(otmp[:, :], po[0:64, :],
                                             prep[:, :])
                        nc.sync.dma_start(
                            out=ot[h // 2][64:128, ib * 512:(ib + 1) * 512],
                            in_=otmp[:, :])
                for lt in range(4):
                    tt = ib * 4 + lt
                    oo = sb_et.tile([128, C], BF16, tag="oo", bufs=3)
                    for ob in range(2):
                        ps = ps_big.tile([128, 512], F32, tag="big")
                        nc.tensor.matmul(
                            ps[:, :], ot[0][:, tt * 128:(tt + 1) * 128],
                            w0[:, ob * 512:(ob + 1) * 512],
                            start=True, stop=False)
                        nc.tensor.matmul(
                            ps[:, :], ot[1][:, tt * 128:(tt + 1) * 128],
                            w1[:, ob * 512:(ob + 1) * 512],
                            start=False, stop=False)
                        nc.tensor.matmul(
                            ps[:, :], ones_t[:1, 0:128],
                            obias[:, ob * 512:(ob + 1) * 512],
                            start=False, stop=True)
                        nc.scalar.copy(oo[:, ob * 512:(ob + 1) * 512],
                                       ps[:, :])
                    nc.sync.dma_start(
                        out=rs_in[ib][lt * 128:(lt + 1) * 128, :],
                        in_=oo[:, :])
                nc.gpsimd.collective_compute(
                    "ReduceScatter", mybir.AluOpType.add,
                    replica_groups=GROUPS,
                    ins=[rs_in[ib][:, :].opt()], outs=[rs_out[ib][:, :].opt()])
                nc.sync.dma_start(out=p_out[ib * 128:(ib + 1) * 128, :],
                                  in_=rs_out[ib][:, :])
            if debug:
                for h in range(HPC):
                    nc.sync.dma_start(out=p_dqa[h], in_=qa[h][:, :])
                    nc.sync.dma_start(out=p_dka[h], in_=ka[h][:, :])
                nc.sync.dma_start(out=p_dmall[:, :], in_=mall[:, :])
                for i in range(2):
                    nc.sync.dma_start(out=p_dot[i], in_=ot[i][:, :])
                for j in range(NT):
                    nc.sync.dma_start(out=p_dva[j], in_=va[j][:, :])

    nc.compile()
    return nc, passA, passB, nA, nB


# ---------------------------------------------------------------------------
# fast path: causal mask + zero biases + overflow-safe scores
# ---------------------------------------------------------------------------
# PE-array tiling: scores as K=64 row-tiled pairs (2 heads concurrent),
# context as M=64 col-tiled pairs, denominators as four concurrent M=1
# col-tiles at partitions 0/32/64/96.  Causal masking via a static 128x128
# bf16 triangle multiplied into et (no mask DMA).  Diagonal column blocks use
# restricted widths.  QKV / out_proj / ReduceScatter chunks are interleaved
# into the pass-B column loop so every engine stays fed.

NCHUNK = 8                       # ReduceScatter chunks (256 query rows each)


def _fast_columns(ib):
    cols = []
    for jt in range(4 * ib):
        cols.append([jt, 512, 0, False])
    for g in range(4):
        cols.append([4 * ib + g, 512 - 128 * g, 128 * g, True])
    return [(jt, w, qoff, diag, i == 0, i == len(cols) - 1)
            for i, (jt, w, qoff, diag) in enumerate(cols)]


def _build_fast_graph():
    nc = bacc.Bacc(num_devices=NCORES)

    p_xT = nc.declare_dram_parameter("xT", [C, T], BF16, isOutput=False)
    p_wqkT = nc.declare_dram_parameter("wqkT", [C, 512], BF16, isOutput=False)
    p_wvT = nc.declare_dram_parameter("wvT", [C, 256], BF16, isOutput=False)
    p_ct = nc.declare_dram_parameter("ct", [128, T], BF16, isOutput=False)
    p_st = nc.declare_dram_parameter("st", [128, T], BF16, isOutput=False)
    p_w0 = nc.declare_dram_parameter("wout0", [128, C], BF16, isOutput=False)
    p_w1 = nc.declare_dram_parameter("wout1", [128, C], BF16, isOutput=False)
    p_tri = nc.declare_dram_parameter("tri", [128, 256], BF16, isOutput=False)
    p_sel = nc.declare_dram_parameter("sel", [97, 256], BF16, isOutput=False)
    p_out = nc.declare_dram_parameter("out", [T // 4, C], BF16, isOutput=True)

    with tile.TileContext(nc) as tc, \
            tc.tile_pool(name="static", bufs=1) as st_pool, \
            tc.tile_pool(name="sdram", bufs=1, space="DRAM") as dr_pool:
        def _t(shape, dtype, name, **k):
            return st_pool.tile(shape, dtype, name=name, tag=name, **k)

        xT = [_t([128, T], BF16, name=f"xT{i}") for i in range(8)]
        wqk = [_t([128, 512], BF16, name=f"wqk{i}") for i in range(8)]
        wv = [_t([128, 256], BF16, name=f"wv{i}") for i in range(8)]
        ct = _t([128, T], BF16, name="ct")
        st = _t([128, T], BF16, name="st")
        w0 = _t([128, C], BF16, name="w0")
        w1 = _t([128, C], BF16, name="w1")
        tri = _t([128, 256], BF16, name="tri")
        sel = _t([97, 256], BF16, name="sel")
        qap = [_t([128, T], BF16, name=f"qap{p}") for p in range(2)]
        kap = [_t([128, T], BF16, name=f"kap{p}") for p in range(2)]
        va = [_t([128, 256], BF16, name=f"vaf{t}") for t in range(NT)]
        ot = [_t([128, T], BF16, name=f"otf{p}") for p in range(2)]
        ones1 = _t([128, 1], BF16, name="ones1")

        nc.vector.memset(ones1[:, :], 1.0)
        for i in range(8):
            nc.sync.dma_start(out=wqk[i][:, :], in_=p_wqkT[i * 128:(i + 1) * 128, :])
        for hf in range(2):
            for i in range(8):
                nc.sync.dma_start(
                    out=xT[i][:, hf * 1024:(hf + 1) * 1024],
                    in_=p_xT[i * 128:(i + 1) * 128, hf * 1024:(hf + 1) * 1024])
            if hf == 0:
                for i in range(8):
                    nc.sync.dma_start(out=wv[i][:, :],
                                      in_=p_wvT[i * 128:(i + 1) * 128, :])
        for sb, pp in ((ct, p_ct), (st, p_st), (tri, p_tri), (sel, p_sel),
                       (w0, p_w0), (w1, p_w1)):
            nc.sync.dma_start(out=sb[:, :], in_=pp[:, :])

        rs_in = [dr_pool.tile([512, C], BF16, name=f"rsf_in{k}",
                              tag=f"rsf_in{k}") for k in range(NIB)]
        rs_out = [dr_pool.tile([128, C], BF16, name=f"rsf_out{k}",
                               tag=f"rsf_out{k}") for k in range(NIB)]

        with (
            tc.tile_pool(name="psS", bufs=2, space="PSUM") as psS_pool,
            tc.tile_pool(name="poP", bufs=1, space="PSUM") as po_pool,
            tc.tile_pool(name="denP", bufs=1, space="PSUM") as den_pool,
            tc.tile_pool(name="mps", bufs=1, space="PSUM") as mps_pool,
            tc.tile_pool(name="raw", bufs=2) as rawp,
            tc.tile_pool(name="rot", bufs=2) as rotp,
            tc.tile_pool(name="tmp", bufs=2) as tmpp,
            tc.tile_pool(name="et", bufs=4) as etp,
            tc.tile_pool(name="densb", bufs=2) as densbp,
            tc.tile_pool(name="prepsb", bufs=2) as prepsbp,
            tc.tile_pool(name="oo", bufs=6) as oop,
        ):
            po = po_pool.tile([128, 1024], F32, tag="po")
            den = den_pool.tile([128, 512], F32, tag="den")
            nc.vector.memset(den[:, :], 0.0)

            def emit_qk_unit(mt, tb):
                ps = mps_pool.tile([128, 512], F32, tag="mps")
                for kc in range(8):
                    nc.tensor.matmul(
                        ps[:, :], wqk[kc][:, mt * 128:(mt + 1) * 128],
                        xT[kc][:, tb * 512:(tb + 1) * 512],
                        start=(kc == 0), stop=(kc == 7))
                rawc = rawp.tile([128, 512], BF16, tag="raw")
                nc.vector.tensor_scalar_add(rawc[:, :], ps[:, :], 0.0)
                rotc = rotp.tile([128, 512], BF16, tag="rot")
                for s in range(2):
                    r = s * 64
                    nc.sync.dma_start(out=rotc[r:r + 32, :],
                                      in_=rawc[r + 32:r + 64, :])
                    nc.sync.dma_start(out=rotc[r + 32:r + 64, :],
                                      in_=rawc[r:r + 32, :])
                tgt = qap[mt] if mt < 2 else kap[mt - 2]
                tsl = tgt[:, tb * 512:(tb + 1) * 512]
                tmp = tmpp.tile([128, 512], BF16, tag="tmp")
                nc.vector.tensor_mul(tmp[:, :], rawc[:, :],
                                     ct[:, tb * 512:(tb + 1) * 512])
                nc.vector.tensor_mul(tsl, rotc[:, :],
                                     st[:, tb * 512:(tb + 1) * 512])
                nc.vector.tensor_add(tsl, tmp[:, :], tsl)

            def emit_v_unit(tt):
                ps = mps_pool.tile([128, 512], F32, tag="mps")
                for kc in range(8):
                    nc.tensor.matmul(
                        ps[:, 0:256], xT[kc][:, tt * 128:(tt + 1) * 128],
                        wv[kc][:, :], start=(kc == 0), stop=(kc == 7))
                nc.vector.tensor_scalar_add(va[tt][:, :], ps[:, 0:256], 0.0)

            oo_live = {}

            def emit_out_unit(ib, lt, half):
                tt = ib * 4 + lt
                ps = mps_pool.tile([128, 512], F32, tag="mps")
                nc.tensor.matmul(ps[:, :], ot[0][:, tt * 128:(tt + 1) * 128],
                                 w0[:, half * 512:(half + 1) * 512],
                                 start=True, stop=False)
                nc.tensor.matmul(ps[:, :], ot[1][:, tt * 128:(tt + 1) * 128],
                                 w1[:, half * 512:(half + 1) * 512],
                                 start=False, stop=True)
                if half == 0:
                    oo_live[tt] = oop.tile([128, 1024], BF16, tag="oo",
                                           name="oo")
                oo = oo_live[tt]
                nc.vector.tensor_scalar_add(
                    oo[:, half * 512:(half + 1) * 512], ps[:, :], 0.0)
                if half == 1:
                    nc.sync.dma_start(
                        out=rs_in[ib][lt * 128:(lt + 1) * 128, :], in_=oo[:, :])
                    del oo_live[tt]

            def emit_collective(k):
                nc.gpsimd.collective_compute(
                    "ReduceScatter", mybir.AluOpType.add,
                    replica_groups=GROUPS,
                    ins=[rs_in[k][:, :].opt()], outs=[rs_out[k][:, :].opt()])
                nc.sync.dma_start(out=p_out[k * 128:(k + 1) * 128, :],
                                  in_=rs_out[k][:, :])

            def emit_col(ib, jt, w, qoff, diag, first, last):
                ets = []
                q0 = ib * 512 + qoff
                q1 = (ib + 1) * 512
                for p in range(2):
                    ps = psS_pool.tile([128, 1024], F32, tag="psS")
                    nc.tensor.matmul(
                        ps[:, 0:w], kap[p][0:64, jt * 128:(jt + 1) * 128],
                        qap[p][0:64, q0:q1], start=True, stop=True)
                    nc.tensor.matmul(
                        ps[:, 512:512 + w],
                        kap[p][64:128, jt * 128:(jt + 1) * 128],
                        qap[p][64:128, q0:q1], start=True, stop=True)
                    et = etp.tile([128, 1024], BF16, tag="et")
                    ps3 = ps[:, :].rearrange("p (two q) -> p two q", two=2)
                    et3 = et[:, :].rearrange("p (two q) -> p two q", two=2)
                    nc.scalar.activation(et3[:, :, 0:w], ps3[:, :, 0:w],
                                         mybir.ActivationFunctionType.Exp)
                    if diag:
                        tri3 = tri[:, :].rearrange("p (two q) -> p two q", two=2)
                        nc.vector.tensor_mul(et3[:, :, 0:128],
                                             et3[:, :, 0:128], tri3[:, :, :])
                    ets.append(et)
                for p in range(2):
                    for s in range(2):
                        h = 2 * p + s
                        nc.tensor.matmul(
                            po[s * 64:s * 64 + 64,
                               512 * p + qoff:512 * p + 512],
                            va[jt][:, h * 64:(h + 1) * 64],
                            ets[p][:, s * 512:s * 512 + w],
                            start=first, stop=last)
                for h in range(4):
                    p, s = h // 2, h % 2
                    nc.tensor.matmul(
                        den[32 * h:32 * h + 1, qoff:512],
                        ones1[:, 0:1], ets[p][:, s * 512:s * 512 + w],
                        start=first, stop=last, tile_position=(0, 32 * h))

            def emit_ib_norm(ib):
                densb = densbp.tile([97, 512], BF16, tag="densb")
                nc.vector.tensor_scalar_add(densb[:, :], den[0:97, :], 0.0)
                for p in range(2):
                    prep = mps_pool.tile([128, 512], F32, tag="mps")
                    nc.tensor.matmul(prep[:, :], sel[:, p * 128:(p + 1) * 128],
                                     densb[:, :], start=True, stop=True)
                    prepsb = prepsbp.tile([128, 512], F32, tag="prepsb")
                    nc.vector.reciprocal_approx_fast(prepsb[:, :], prep[:, :])
                    nc.vector.tensor_mul(
                        ot[p][:, ib * 512:(ib + 1) * 512],
                        po[:, 512 * p:512 * p + 512], prepsb[:, :])

            for mt in range(4):
                emit_qk_unit(mt, 0)
            for tt in range(4):
                emit_v_unit(tt)

            # All DMA-dependent units (qk rot DMAs, v) are front-loaded into
            # ib0/ib1; collectives are issued only from ib2 on, when pass B
            # no longer depends on fresh DMAs the collective would block.
            unit_plan = {
                0: [("qk", mt, 1) for mt in range(4)]
                   + [("v", 4 + i) for i in range(4)],
                1: [("qk", mt, 2) for mt in range(4)]
                   + [("qk", mt, 3) for mt in range(4)]
                   + [("v", 8 + i) for i in range(8)]
                   + [("out", 0, lt, hf) for lt in range(4) for hf in range(2)],
                2: [("out", 1, lt, hf) for lt in range(4) for hf in range(2)],
                3: [("out", 2, lt, hf) for lt in range(4) for hf in range(2)],
            }
            cc_plan = {2: {1: 0, 8: 1}, 3: {7: 2}}   # ib -> {col idx: chunk}

            for ib in range(NIB):
                units = unit_plan[ib]
                ccs = cc_plan.get(ib, {})
                cols = _fast_columns(ib)
                spread = len(cols) if ib < 2 else 6
                emitted = 0
                for ci, col in enumerate(cols):
                    emit_col(ib, *col)
                    if ci in ccs:
                        emit_collective(ccs[ci])
                    want = min(len(units), (ci + 1) * len(units) // spread)
                    while emitted < want:
                        u = units[emitted]
                        emitted += 1
                        if u[0] == "qk":
                            emit_qk_unit(u[1], u[2])
                        elif u[0] == "v":
                            emit_v_unit(u[1])
                        else:
                            emit_out_unit(u[1], u[2], u[3])
                emit_ib_norm(ib)
            for lt in range(4):
                for half in range(2):
                    emit_out_unit(3, lt, half)
            emit_collective(3)

    nc.compile()
    return nc


def _fast_static_tables():
    t1 = (np.arange(128)[None, :] >= np.arange(128)[:, None]).astype(np.float32)
    tri = np.concatenate([t1, t1], axis=1).astype(NPBF16)
    sel = np.zeros((97, 256), dtype=np.float32)
    for p in range(2):
        sel[64 * p, p * 128:p * 128 + 64] = 1.0
        sel[64 * p + 32, p * 128 + 64:p * 128 + 128] = 1.0
    return tri, sel.astype(NPBF16)


def _prep_core_fast(inputs, c):
    b, hg = divmod(c, 4)
    f0 = hg * FPC

    x = inputs["x"][b]
    xT = np.ascontiguousarray(x.T).astype(NPBF16)

    scale = 1.0 / np.sqrt(D)
    qw = inputs["qkv_weight"]
    qs = qw[f0:f0 + FPC] * scale
    ks = qw[C + f0:C + f0 + FPC]
    vs = qw[2 * C + f0:2 * C + f0 + FPC]
    wqkT = np.ascontiguousarray(np.concatenate([qs, ks], 0).T).astype(NPBF16)
    wvT = np.ascontiguousarray(vs.T).astype(NPBF16)

    wout = inputs["out_proj_weight"]
    wsh = np.ascontiguousarray(wout[:, f0:f0 + FPC].T)
    w0 = wsh[0:128].astype(NPBF16)
    w1 = wsh[128:256].astype(NPBF16)

    ctf, stf = _rope_tables(inputs["cos"], inputs["sin"])
    tri, sel = _fast_static_tables()
    return dict(xT=xT, wqkT=wqkT, wvT=wvT, ct=ctf, st=stf,
                wout0=w0, wout1=w1, tri=tri, sel=sel)


def _fast_path_ok(inputs):
    if not (np.all(np.asarray(inputs["qkv_bias"]) == 0)
            and np.all(np.asarray(inputs["out_proj_bias"]) == 0)):
        return False
    am = np.asarray(inputs["attn_mask"], dtype=np.float32)
    causal = np.triu(np.full((T, T), -1e9, dtype=np.float32), k=1)
    if not np.array_equal(am, np.broadcast_to(causal[None, None],
                                              (B, 1, T, T))):
        return False
    x = np.asarray(inputs["x"], dtype=np.float32).reshape(-1, C)
    w = np.asarray(inputs["qkv_weight"], dtype=np.float32)
    q = x @ w[:C].T
    k = x @ w[C:2 * C].T
    qn = np.linalg.norm(q.reshape(-1, H, D), axis=2).max(axis=0)
    kn = np.linalg.norm(k.reshape(-1, H, D), axis=2).max(axis=0)
    return bool((qn * kn).max() / np.sqrt(D) < 70.0)


def _run_fast(inputs, trace=False):
    if "fast" not in _GRAPH_CACHE:
        _GRAPH_CACHE["fast"] = _build_fast_graph()
    nc = _GRAPH_CACHE["fast"]
    in_maps = [_prep_core_fast(inputs, c) for c in range(NCORES)]
    res = run_bass_kernel_spmd(nc, in_maps, list(range(NCORES)), trace=trace)
    _run.last_exec_time_ns = res.exec_time_ns

    out = np.empty((B, T, C), dtype=np.float32)
    for c in range(NCORES):
        b, g = divmod(c, 4)
        sh = np.asarray(res.results[c]["out"], dtype=np.float32)
        for ib in range(NIB):
            out[b, ib * 512 + g * 128:ib * 512 + (g + 1) * 128, :] = \
                sh[ib * 128:(ib + 1) * 128]
    return out


# ---------------------------------------------------------------------------
# host side
# ---------------------------------------------------------------------------

_GRAPH_CACHE = {}


def _rope_tables(cos, sin):
    cosT = np.ascontiguousarray(cos.T.astype(np.float32))    # [64, T]
    sinT = np.ascontiguousarray(sin.T.astype(np.float32))
    sin_r = np.concatenate([-sinT[0:32], sinT[32:64]], axis=0)   # rotate sign
    ct = np.tile(cosT, (2, 1))
    st = np.tile(sin_r, (2, 1))
    return ct.astype(NPBF16), st.astype(NPBF16)


def _pack_masks(attn_mask, b, passA, passB, nA, nB):
    mb = attn_mask[b, 0]                                     # [T, T] f32
    mA = np.zeros((max(nA, 1), 128, 128), dtype=np.float32)
    idx = 0
    for it in range(NT):
        for (j0, njt, masked) in passA[it]:
            for off in masked:
                jt = j0 + off
                mA[idx] = mb[it * 128:(it + 1) * 128, jt * 128:(jt + 1) * 128]
                idx += 1
    mB = np.zeros((max(nB, 1), 128, 512), dtype=np.float32)
    idx = 0
    for ib in range(NIB):
        for (jt, msk_flag) in passB[ib]:
            if msk_flag:
                mB[idx] = mb[ib * 512:(ib + 1) * 512,
                             jt * 128:(jt + 1) * 128].T
                idx += 1
    return mA, mB


def _prep_core(inputs, c, passA, passB, nA, nB, mask_cache):
    b, hg = divmod(c, 4)
    f0 = hg * FPC

    x = inputs["x"][b]                                       # [T, C]
    xT = np.ascontiguousarray(x.T).astype(NPBF16)            # [C, T]

    scale = 1.0 / np.sqrt(D)                    # folded into q weights/bias
    qw = inputs["qkv_weight"]                                # [3C, C]
    qs = qw[f0:f0 + FPC] * scale
    ks = qw[C + f0:C + f0 + FPC]
    vs = qw[2 * C + f0:2 * C + f0 + FPC]
    wqkT = np.ascontiguousarray(np.concatenate([qs, ks], 0).T).astype(NPBF16)
    wvT = np.ascontiguousarray(vs.T).astype(NPBF16)

    qb = inputs["qkv_bias"]
    qkb = np.concatenate([qb[f0:f0 + FPC] * scale,
                          qb[C + f0:C + f0 + FPC]])[None, :].astype(NPBF16)
    vb = qb[2 * C + f0:2 * C + f0 + FPC][None, :].astype(NPBF16)

    wout = inputs["out_proj_weight"]                         # [C, C]
    wsh = np.ascontiguousarray(wout[:, f0:f0 + FPC].T)       # [256, C]
    w0 = wsh[0:128].astype(NPBF16)
    w1 = wsh[128:256].astype(NPBF16)
    ob = (inputs["out_proj_bias"] if hg == 0
          else np.zeros_like(inputs["out_proj_bias"]))[None, :].astype(NPBF16)

    if b not in mask_cache:
        mask_cache[b] = _pack_masks(inputs["attn_mask"], b, passA, passB,
                                    nA, nB)
    mA, mB = mask_cache[b]

    ct, st = _rope_tables(inputs["cos"], inputs["sin"])

    return dict(xT=xT, wqkT=wqkT, wvT=wvT, qkb=qkb, vb=vb, ct=ct, st=st,
                wout0=w0, wout1=w1, obias=ob, maskA=mA, maskB=mB)


def _score_bound_safe(inputs, attn_mask):
    '''True if exp(S + mask) cannot overflow/underflow without row-max
    subtraction.  RoPE is a per-pair rotation, so L2 norms of q/k rows are
    preserved and max|S| <= max_i|q_i| * max_j|k_j| / sqrt(D) per head.'''
    if (attn_mask <= -1e8).all(axis=3).any():
        return False                      # fully-masked rows need the m path
    x = np.asarray(inputs["x"], dtype=np.float32).reshape(-1, C)
    w = np.asarray(inputs["qkv_weight"], dtype=np.float32)
    b = np.asarray(inputs["qkv_bias"], dtype=np.float32)
    q = x @ w[:C].T + b[:C]
    k = x @ w[C:2 * C].T + b[C:2 * C]
    qn = np.linalg.norm(q.reshape(-1, H, D), axis=2).max(axis=0)   # per head
    kn = np.linalg.norm(k.reshape(-1, H, D), axis=2).max(axis=0)
    bound = (qn * kn).max() / np.sqrt(D) + max(attn_mask.max(), 0.0)
    return bound < 70.0


def _run(inputs, trace=False):
    if _fast_path_ok(inputs):
        return _run_fast(inputs, trace=trace)
    attn_mask = np.asarray(inputs["attn_mask"], dtype=np.float32)
    flags = _analyze_mask(attn_mask)
    mfree = _score_bound_safe(inputs, attn_mask)
    key = (flags.tobytes(), mfree)
    if key not in _GRAPH_CACHE:
        _GRAPH_CACHE[key] = _build_graph(flags, mfree=mfree)
    nc, passA, passB, nA, nB = _GRAPH_CACHE[key]

    mask_cache = {}
    in_maps = [_prep_core(inputs, c, passA, passB, nA, nB, mask_cache)
               for c in range(NCORES)]
    res = run_bass_kernel_spmd(nc, in_maps, list(range(NCORES)), trace=trace)
    _run.last_exec_time_ns = res.exec_time_ns

    out = np.empty((B, T, C), dtype=np.float32)
    for c in range(NCORES):
        b, r = divmod(c, 4)
        sh = np.asarray(res.results[c]["out"], dtype=np.float32)
        for ib in range(NIB):
            out[b, ib * 512 + r * 128:ib * 512 + (r + 1) * 128, :] = \
                sh[ib * 128:(ib + 1) * 128]
    return out


_run.last_exec_time_ns = None


def kernel(**inputs):
    return _run(inputs, trace=False)

